# revision 35
# baseline (speedup 1.0000x reference)
# Self-contained Trainium2 Bass kernel for the deformable-conv problem.
# kernel(**inputs) takes FULL unsharded inputs, shards batch across 8 cores,
# runs one Bass program SPMD, and reassembles the full output.
#
# v2: quad-table entries reordered to (dx, dy, c) so the table builds from an
# interleaved row-pair image with 512B-contiguous runs; WM weight replication
# via stride-0-source DRAM->SBUF HWDGE DMAs (2/call).
# v3: call tiles changed from 4-rows-x-128-cols to 16-rows-x-32-cols so the
# SWDGE index buffer (16-partition-wrapped, 8x core-replicated) is built from
# XBAR-transposed index planes with >=64B-run DMAs instead of the old
# 2B-granular replication chain (8 x 307us serial on the sync queue, the
# dominant cost in v2).  Weight scatter likewise rebuilt with merged (q,n)
# dims at 32B runs.  The per-tap weight multiply moves gpsimd->DVE so gpsimd
# only issues gathers (no more LOAD_LIB/UNLOAD_LIB thrash between the SWDGE
# lib and the TT lib).
import os

import numpy as np

import concourse.bacc as bacc
import concourse.bass as bass
import concourse.mybir as mybir
import concourse.tile as tile
from concourse.tile import add_dep_helper
from concourse.bass_utils import run_bass_kernel_spmd

f32 = mybir.dt.float32
f16 = mybir.dt.float16
i16 = mybir.dt.int16
i32 = mybir.dt.int32
OP = mybir.AluOpType
P = 128
N9 = 9
C = 64
F = 64
H = 128
W = 128
NPIX = H * W
PXB = 512
DI = PXB // P
NCALLS = NPIX // PXB
CALL = PXB * N9
SPC = CALL // 16
HCH = 64
NCH = H // HCH
MCH = HCH * N9
MFREE = H * N9
XCOL_RANK = H + 1
CORNER_RANK = H + 2
NRANKS = H + 3
NSLOTS = NRANKS * 128
GIDX = PXB
GSP = GIDX // 16
NCORES = 8


def _build_kernel(tc, outs, ins, dbg=None):
    nc = tc.nc
    x_in, xoff, kdup, mi_in, mr_in, jp1_in = ins
    out64 = outs

    with tc.tile_pool(name="dram", bufs=1, space="DRAM") as dpool:
        qt_d = dpool.tile([NSLOTS * 256], f16)
        w4_d = dpool.tile([NCALLS * 4 * CALL], f16)
        idx16_d = dpool.tile([16 * SPC * NCALLS], i16)
        qtt, qto = qt_d[:].tensor, qt_d[:].offset

        def S(name, shape, dtype):
            return nc.alloc_sbuf_tensor("sb_" + name, shape, dtype).ap()

        idxw = S("idxw", [P, SPC * NCALLS], i16)
        idxh2 = S("idxh2", [P, MFREE], i16)
        G_ring = [S(f"Gr{k}", [P, 2, PXB], f16) for k in range(8)]
        S_ring = [S(f"Sr{k}", [P, 2, PXB], f16) for k in range(8)]
        WM_ring = [S(f"WMr{k}", [P, 2, CALL], f16) for k in range(2)]
        ps_ring = [nc.alloc_psum_tensor(f"psr{k}", [F, PXB], f32).ap()
                   for k in range(4)]
        ob_ring = [S(f"obr{k}", [F, PXB], f32) for k in range(3)]
        kd = S("kd", [P, N9 * F], f16)
        jp1 = S("jp1", [P, 1], f32)
        w4h = S("w4h", [P, 4 * MFREE], f16)
        idxh = S("idxh", [P, MFREE], i16)
        kdsrc = bass.AP(kdup[:].tensor, kdup[:].offset,
                        [[F, P], [P * F, N9], [1, F]])
        nc.sync.dma_start(out=kd[:], in_=kdsrc)
        nc.sync.dma_start(out=jp1[:], in_=jp1_in[:])

        # Hoist chunk-0 Stage-B input loads ahead of Stage A so the DVE
        # weight/index pipeline starts at ~10us instead of queueing its
        # loads behind the (long) table-build DMAs.
        _tc = {}

        def T(name, dtype=f32):
            if name not in _tc:
                _tc[name] = S(name, [P, MCH], dtype)
            return _tc[name]

        def load_chunk(ich):
            ibase = ich * HCH
            xo = T(f"xo{ich}")
            mi = T(f"mi{ich}")
            mr = T(f"mr{ich}")
            xosrc = bass.AP(xoff[:].tensor, xoff[:].offset + ibase * W * N9,
                            [[N9, W], [W * N9, HCH], [1, N9]])
            nc.sync.dma_start(out=xo[:], in_=xosrc)
            nc.sync.dma_start(out=mi[:], in_=mi_in[:, ibase * N9:(ibase + HCH) * N9])
            nc.sync.dma_start(out=mr[:], in_=mr_in[:, ibase * N9:(ibase + HCH) * N9])

        load_chunk(0)
        load_chunk(1)

        # ---- Stage A: fp16 image in SBUF + quad table in DRAM ----
        # Entry element order is (dy, dx, c): elems [0:128] = dy=0 pair
        # [X(y0-1,x0-1), X(y0-1,x0)] — a 256B CONTIGUOUS run of one xh row —
        # elems [128:256] = dy=1 pair from the next row.  After the
        # transpose-gather, partitions = (dx, c) and the free slot = dy; kd
        # is duplicated symmetrically so the matmul contraction is
        # unchanged.  This order builds the table STRAIGHT from xh (no
        # interleaved xq intermediate, whose build was a 170us serial DMA).
        xh = S("xh", [H, W * C], f16)
        XCH = W * C // 4
        xf = S("xf", [H, XCH], f32)
        for ch in range(4):
            xsrc = bass.AP(x_in[:].tensor, x_in[:].offset + ch * XCH,
                           [[W * C, H], [1, XCH]])
            nc.sync.dma_start(out=xf[:], in_=xsrc)
            nc.scalar.copy(out=xh[:, ch * XCH:(ch + 1) * XCH], in_=xf[:])
        zt = S("zt", [P, 512], f16)
        nc.vector.memset(zt[:], 0.0)
        ztp = zt[:].ap[0][0]
        zsrc = lambda rows, elems: bass.AP(zt[:].tensor, zt[:].offset,
                                           [[ztp, rows], [1, elems]])
        xpitch = xh[:].ap[0][0]
        xbase = xh[:].offset
        # Main window (y0 0..127, x0 1..127), split by dy half and by y0
        # half across both HWDGE queues (4 data DMAs, 256B runs from xh).
        for yh in range(2):
            y0a = yh * 64
            # dy=0 half <- xh row y0-1 (y0=0 row is zero-filled below)
            na = 63 if yh == 0 else 64
            ya = 1 if yh == 0 else 64
            nc.sync.dma_start(
                out=bass.AP(qtt, qto + ya * 128 * 256 + 256,
                            [[128 * 256, na], [256, W - 1], [1, 128]]),
                in_=bass.AP(xh[:].tensor, xbase + (ya - 1) * xpitch,
                            [[xpitch, na], [C, W - 1], [1, 128]]))
            # dy=1 half <- xh row y0
            nc.scalar.dma_start(
                out=bass.AP(qtt, qto + y0a * 128 * 256 + 256 + 128,
                            [[128 * 256, 64], [256, W - 1], [1, 128]]),
                in_=bass.AP(xh[:].tensor, xbase + y0a * xpitch,
                            [[xpitch, 64], [C, W - 1], [1, 128]]))
        # y0=0, dy=0 halves are X(-1,.) = 0 (x0 1..127)
        nc.sync.dma_start(out=bass.AP(qtt, qto + 256, [[256, W - 1], [1, 128]]),
                          in_=zsrc(W - 1, 128))
        # x0=0 column (y0 0..127): dx=0 (padded col 0) zero quarters at
        # [0:64] and [128:192]; dx=1 <- xh col 0: dy0 [64:128] from row
        # y0-1 (zero for y0=0), dy1 [192:256] from row y0.
        nc.scalar.dma_start(out=bass.AP(qtt, qto, [[128 * 256, 128], [1, 64]]),
                            in_=zsrc(128, 64))
        nc.sync.dma_start(out=bass.AP(qtt, qto + 128, [[128 * 256, 128], [1, 64]]),
                          in_=zsrc(128, 64))
        nc.scalar.dma_start(
            out=bass.AP(qtt, qto + 128 * 256 + 64, [[128 * 256, 127], [1, 64]]),
            in_=bass.AP(xh[:].tensor, xbase, [[xpitch, 127], [1, 64]]))
        nc.sync.dma_start(out=bass.AP(qtt, qto + 64, [[1, 1], [1, 64]]),
                          in_=zsrc(1, 64))
        nc.scalar.dma_start(
            out=bass.AP(qtt, qto + 192, [[128 * 256, 128], [1, 64]]),
            in_=bass.AP(xh[:].tensor, xbase, [[xpitch, 128], [1, 64]]))
        # rank y0=128 (slots 128*128+x0): dy=0 <- xh row 127, dy=1 = zero.
        nc.sync.dma_start(
            out=bass.AP(qtt, qto + (128 * 128 + 1) * 256, [[256, W - 1], [1, 128]]),
            in_=bass.AP(xh[:].tensor, xbase + 127 * xpitch,
                        [[xpitch, 1], [C, W - 1], [1, 128]]))
        # x0=0 of rank 128: [0:64]=0, [64:128]=xh[127,0:64]
        nc.scalar.dma_start(
            out=bass.AP(qtt, qto + 128 * 128 * 256, [[1, 1], [1, 64]]),
            in_=zsrc(1, 64))
        nc.sync.dma_start(
            out=bass.AP(qtt, qto + 128 * 128 * 256 + 64, [[1, 1], [1, 64]]),
            in_=bass.AP(xh[:].tensor, xbase + 127 * xpitch,
                        [[xpitch, 1], [1, 64]]))
        nc.scalar.dma_start(
            out=bass.AP(qtt, qto + 128 * 128 * 256 + 128, [[256, 128], [1, 128]]),
            in_=zsrc(128, 128))
        # xcol rank (x0=128, slots 129*128+y0): dx=0 <- xh col 127; dx=1
        # (padded col 129) zero quarters.
        nc.sync.dma_start(
            out=bass.AP(qtt, qto + (XCOL_RANK * 128 + 1) * 256,
                        [[256, 127], [1, 64]]),
            in_=bass.AP(xh[:].tensor, xbase + (W - 1) * C,
                        [[xpitch, 127], [1, 64]]))
        nc.scalar.dma_start(
            out=bass.AP(qtt, qto + XCOL_RANK * 128 * 256, [[1, 1], [1, 64]]),
            in_=zsrc(1, 64))
        nc.sync.dma_start(
            out=bass.AP(qtt, qto + XCOL_RANK * 128 * 256 + 128,
                        [[256, 128], [1, 64]]),
            in_=bass.AP(xh[:].tensor, xbase + (W - 1) * C,
                        [[xpitch, 128], [1, 64]]))
        nc.scalar.dma_start(
            out=bass.AP(qtt, qto + XCOL_RANK * 128 * 256 + 64,
                        [[256, 128], [1, 64]]),
            in_=zsrc(128, 64))
        nc.sync.dma_start(
            out=bass.AP(qtt, qto + XCOL_RANK * 128 * 256 + 192,
                        [[256, 128], [1, 64]]),
            in_=zsrc(128, 64))
        # corner (x0=128, y0=128): [0:64] = X(127,127), [64:256] = 0
        nc.scalar.dma_start(
            out=bass.AP(qtt, qto + CORNER_RANK * 128 * 256, [[1, 1], [1, C]]),
            in_=bass.AP(xh[:].tensor, xbase + 127 * xpitch + (W - 1) * C,
                        [[xpitch, 1], [1, C]]))
        nc.sync.dma_start(
            out=bass.AP(qtt, qto + CORNER_RANK * 128 * 256 + 64, [[1, 1], [1, 192]]),
            in_=zsrc(1, 192))

        # ---- Stage B: indices & weights ----
        w4h4 = w4h[:].rearrange("p (q n i) -> p q n i", q=4, n=N9, i=H)
        idxh3 = idxh[:].rearrange("p (n i) -> p n i", n=N9, i=H)

        for ich in range(NCH):
            ibase = ich * HCH
            xo = T(f"xo{ich}")
            mi = T(f"mi{ich}")
            mr = T(f"mr{ich}")

            def side(pre, base_is_j):
                # rel/ti/tf/hi/lo are dead once side() returns — share one
                # scratch set between the x and y passes (SBUF pressure).
                rel = T("srel")
                if base_is_j:
                    nc.vector.tensor_scalar(out=rel[:], in0=xo[:], scalar1=jp1[:, 0:1],
                                            scalar2=None, op0=OP.add)
                else:
                    nc.vector.tensor_tensor(out=rel[:], in0=xo[:], in1=mi[:], op=OP.add)
                nc.vector.tensor_tensor(out=rel[:], in0=rel[:], in1=mr[:], op=OP.add)
                ti = T("sti", i32)
                tf = T("stf")
                nc.vector.tensor_copy(out=ti[:], in_=rel[:])
                nc.vector.tensor_copy(out=tf[:], in_=ti[:])
                corr = T(pre + "corr")
                nc.vector.tensor_tensor(out=corr[:], in0=tf[:], in1=rel[:], op=OP.is_gt)
                nc.vector.tensor_tensor(out=tf[:], in0=tf[:], in1=corr[:], op=OP.subtract)
                r0 = tf
                dim = W if base_is_j else H
                c0 = T(pre + "c0")
                nc.vector.tensor_scalar(out=c0[:], in0=r0[:], scalar1=0.0,
                                        scalar2=float(dim + 1), op0=OP.max, op1=OP.min)
                c1 = T(pre + "c1")
                nc.vector.tensor_scalar(out=c1[:], in0=r0[:], scalar1=1.0, scalar2=0.0,
                                        op0=OP.add, op1=OP.max)
                nc.vector.tensor_scalar(out=c1[:], in0=c1[:], scalar1=float(dim + 1),
                                        scalar2=None, op0=OP.min)
                nc.vector.tensor_tensor(out=c1[:], in0=c1[:], in1=rel[:], op=OP.subtract)
                w0 = c1
                nc.vector.tensor_tensor(out=c0[:], in0=rel[:], in1=c0[:], op=OP.subtract)
                w1 = c0
                hi = T("shi")
                nc.vector.tensor_scalar(out=hi[:], in0=r0[:], scalar1=float(dim + 1),
                                        scalar2=None, op0=OP.is_ge)
                lo = T("slo")
                nc.vector.tensor_scalar(out=lo[:], in0=r0[:], scalar1=-1.0,
                                        scalar2=None, op0=OP.is_le)
                nc.vector.tensor_tensor(out=hi[:], in0=w0[:], in1=hi[:], op=OP.mult)
                t0 = hi
                nc.vector.tensor_tensor(out=lo[:], in0=w1[:], in1=lo[:], op=OP.mult)
                t1 = lo
                nc.vector.tensor_tensor(out=w0[:], in0=w0[:], in1=t0[:], op=OP.subtract)
                nc.vector.tensor_tensor(out=w0[:], in0=w0[:], in1=t1[:], op=OP.add)
                nc.vector.tensor_tensor(out=w1[:], in0=w1[:], in1=t1[:], op=OP.subtract)
                nc.vector.tensor_tensor(out=w1[:], in0=w1[:], in1=t0[:], op=OP.add)
                nc.vector.tensor_scalar(out=corr[:], in0=r0[:], scalar1=0.0,
                                        scalar2=float(dim), op0=OP.max, op1=OP.min)
                return w0, w1, corr

            B0, B1, bx = side("x", True)
            A0, A1, by = side("y", False)

            prod = T("prod")
            for qi, (ay, bw) in enumerate(((A0, B0), (A0, B1), (A1, B0), (A1, B1))):
                nc.vector.tensor_tensor(out=prod[:], in0=ay[:], in1=bw[:], op=OP.mult)
                nc.vector.tensor_copy(
                    out=w4h4[:, qi, :, ibase:ibase + HCH],
                    in_=prod[:].rearrange("p (i n) -> p n i", i=HCH, n=N9))

            m128 = T("srel")
            nc.vector.tensor_scalar(out=m128[:], in0=bx[:], scalar1=float(W),
                                    scalar2=None, op0=OP.is_ge)
            idxa = T("stf")
            nc.vector.tensor_scalar(out=idxa[:], in0=by[:], scalar1=128.0,
                                    scalar2=None, op0=OP.mult)
            nc.vector.tensor_tensor(out=idxa[:], in0=idxa[:], in1=bx[:], op=OP.add)
            idxb = T("shi")
            nc.vector.tensor_scalar(out=idxb[:], in0=by[:],
                                    scalar1=float(XCOL_RANK * 128),
                                    scalar2=None, op0=OP.add)
            nc.vector.tensor_tensor(out=idxb[:], in0=idxb[:], in1=idxa[:], op=OP.subtract)
            nc.vector.tensor_tensor(out=idxb[:], in0=idxb[:], in1=m128[:], op=OP.mult)
            nc.vector.tensor_tensor(out=idxa[:], in0=idxa[:], in1=idxb[:], op=OP.add)
            nc.vector.tensor_copy(
                out=idxh3[:, :, ibase:ibase + HCH],
                in_=idxa[:].rearrange("p (i n) -> p n i", i=HCH, n=N9))

        # Call cc = ig*4+cg covers rows [16ig,16ig+16) x cols [32cg,32cg+32);
        # SWDGE enumerates px = 16*jl + iq (iq = idx-partition-within-16,
        # jl = free slot), so pixel (16ig+iq, 32cg+jl) sits at px.
        #
        # idxw path: XBAR-transpose the 9 idx planes [p=j, i] -> idxh2
        # [p=i, j], scatter to DRAM in the 16-wrapped order (64B runs),
        # then 8 fully-contiguous replication DMAs (one per gpsimd core
        # group, 16 descriptors x 18KB each).
        w4p = w4h[:].ap[0][0]
        idxp = idxh[:].ap[0][0]
        i2p = idxh2[:].ap[0][0]
        for n in range(N9):
            eng = nc.sync if n % 2 else nc.scalar
            eng.dma_start_transpose(
                out=idxh2[:, n * H:(n + 1) * H],
                in_=idxh[:, n * H:(n + 1) * H])
        for ig in range(8):
            for cg in range(4):
                eng = nc.sync if (ig * 4 + cg) % 2 else nc.scalar
                src = bass.AP(idxh2[:].tensor,
                              idxh2[:].offset + 16 * ig * i2p + 32 * cg,
                              [[i2p, 16], [H, N9], [1, 32]])
                dst = bass.AP(idx16_d[:].tensor,
                              idx16_d[:].offset + (ig * 4 + cg) * SPC,
                              [[SPC * NCALLS, 16], [32, N9], [1, 32]])
                eng.dma_start(out=dst, in_=src)
        ipitch = idxw[:].ap[0][0]
        for rep in range(8):
            iwdst = bass.AP(idxw[:].tensor, idxw[:].offset + rep * 16 * ipitch,
                            [[ipitch, 16], [1, SPC * NCALLS]])
            iwsrc = bass.AP(idx16_d[:].tensor, idx16_d[:].offset,
                            [[SPC * NCALLS, 16], [1, SPC * NCALLS]])
            nc.sync.dma_start(out=iwdst, in_=iwsrc)
        # w4 scatter: dst elem = cc*4*CALL + (2*dy+dx)*CALL + n*PXB + 16*jl
        # + iq; (q4,n) merge to one stride-512 level on both sides.
        for ig in range(8):
            for cg in range(4):
                eng = nc.sync if (ig * 4 + cg) % 2 else nc.scalar
                src = bass.AP(w4h[:].tensor,
                              w4h[:].offset + 32 * cg * w4p + 16 * ig,
                              [[w4p, 32], [H, 36], [1, 16]])
                dst = bass.AP(w4_d[:].tensor,
                              w4_d[:].offset + (ig * 4 + cg) * 4 * CALL,
                              [[16, 32], [PXB, 36], [1, 16]])
                eng.dma_start(out=dst, in_=src)
        if dbg is not None:
            nc.sync.dma_start(out=dbg["idxw"][:], in_=idxw[:])
            nc.scalar.dma_start(out=dbg["idxh2"][:], in_=idxh2[:])
            nc.sync.dma_start(out=dbg["idxh"][:], in_=idxh[:])
            nc.scalar.dma_start(out=dbg["w4h"][:], in_=w4h[:])

        # ---- main loop ----
        # WM[p=(dy,c), dx, n*PXB+px] = w(dy,dx,n,px), built by a single
        # stride-0-source DRAM->SBUF DMA per dy half (64 replicated reads
        # of the same 9216B row -> 128 contiguous descriptors). No
        # SBUF->SBUF DMA in the loop, so gathers need no serialization.
        qtv = qt_d[:].rearrange("(s e) -> s e", s=NSLOTS, e=256)
        ob_insts = [None, None, None, None]
        use_dve = os.environ.get("DEFCONV_TT_ENG", "vector") != "gpsimd"

        def emit_tt_mm(cc, n, k, xbar_margin_gi):
            # TT for tap k; on DVE it additionally waits for gather k+1's
            # DMA so the XBAR transpose-write of gather k has fully landed
            # before DVE touches it (gpsimd consumers are slow enough to
            # never expose that window; DVE is not).
            WM = WM_ring[cc % 2]
            wmp = WM[:].ap[0][0]
            Sg = S_ring[k % 8]
            G = G_ring[k % 8]
            wmin = bass.AP(WM[:].tensor, WM[:].offset + n * PXB,
                           [[wmp, P], [CALL, 2], [1, PXB]])
            eng = nc.vector if (use_dve and xbar_margin_gi is not None) \
                else nc.gpsimd
            tti = eng.tensor_tensor(out=Sg[:], in0=G[:], in1=wmin, op=OP.mult)
            if use_dve and xbar_margin_gi is not None:
                add_dep_helper(tti.ins, xbar_margin_gi.ins, True, "xbar margin")
            if dbg is not None and "sdump" in dbg:
                st, so = dbg["sdump"][:].tensor, dbg["sdump"][:].offset
                nc.scalar.dma_start(
                    out=bass.AP(st, so + k * P * 1024, [[1024, P], [1, 1024]]),
                    in_=Sg[:])
            ps = ps_ring[cc % 4]
            lhsT = kd[:, n * F:(n + 1) * F]
            for dx in range(2):
                mmi = nc.tensor.matmul(
                    ps[:], lhsT, Sg[:, dx, :],
                    start=(n == 0 and dx == 0),
                    stop=(n == N9 - 1 and dx == 1))
                if n == 0 and dx == 0 and ob_insts[cc % 4] is not None:
                    add_dep_helper(mmi.ins, ob_insts[cc % 4].ins, True, "psum reuse")
            if n == N9 - 1:
                ob = ob_ring[cc % 3]
                obi = nc.scalar.copy(out=ob[:], in_=ps[:])
                ob_insts[cc % 4] = obi
                nc.sync.dma_start(out=out64[:, cc * PXB:(cc + 1) * PXB], in_=ob[:])

        pend = None
        for cc in range(NCALLS):
            WM = WM_ring[cc % 2]
            wmp = WM[:].ap[0][0]
            # partitions are (dx, c) now and the gather free half = dy:
            # partition half dxh holds q4 = 2*dy + dxh at free half dy,
            # so the src walks dy with stride 2*CALL.
            for dxh in range(2):
                wmdst = bass.AP(WM[:].tensor, WM[:].offset + dxh * C * wmp,
                                [[wmp, C], [CALL, 2], [1, CALL]])
                wmsrc = bass.AP(w4_d[:].tensor,
                                w4_d[:].offset + (cc * 4 + dxh) * CALL,
                                [[0, C], [2 * CALL, 2], [1, CALL]])
                nc.scalar.dma_start(out=wmdst, in_=wmsrc)
            for n in range(N9):
                k = cc * N9 + n
                gi = nc.gpsimd.dma_gather(
                    G_ring[k % 8][:], qtv,
                    idxw[:, cc * SPC + n * GSP: cc * SPC + (n + 1) * GSP],
                    num_idxs=GIDX, num_idxs_reg=GIDX, elem_size=256,
                    transpose=True,
                    queue_num=k % int(os.environ.get("DEFCONV_NQ", "1")))
                if dbg is not None and "gdump" in dbg:
                    gt, go = dbg["gdump"][:].tensor, dbg["gdump"][:].offset
                    nc.sync.dma_start(
                        out=bass.AP(gt, go + k * P * 1024,
                                    [[1024, P], [1, 1024]]),
                        in_=G_ring[k % 8][:])
                if pend is not None:
                    emit_tt_mm(*pend, xbar_margin_gi=gi)
                pend = (cc, n, k)
        emit_tt_mm(*pend, xbar_margin_gi=None)


def _make_consts(kernel_np):
    k9 = kernel_np.reshape(N9, C, F)
    kdup = np.concatenate([k9, k9], axis=1).astype(np.float16)
    ii = np.repeat(np.arange(H, dtype=np.float32) + 1.0, N9)
    R = np.tile(np.arange(-1, 2, dtype=np.float32), 3)
    rr = np.tile(R, H)
    mi = np.broadcast_to(ii, (P, H * N9)).copy()
    mr = np.broadcast_to(rr, (P, H * N9)).copy()
    jp1 = (np.arange(P, dtype=np.float32) + 1.0).reshape(P, 1)
    return kdup, mi, mr, jp1


_CACHE = {}


def _get_nc():
    if "nc" in _CACHE:
        return _CACHE["nc"]
    debug = bool(os.environ.get("DEFCONV_DEBUG"))
    # num_swdge_queues MUST stay 1: concurrent SWDGE transpose-gather DMAs
    # on >=2 queues corrupt the gathered data (shared XBAR transpose-RX
    # hazard, reproduced at NQ=2 and NQ=4 with px%16-lane-granular garbage).
    nq = int(os.environ.get("DEFCONV_NQ", "1"))
    nc = bacc.Bacc("TRN2", target_bir_lowering=False, debug=False,
                   num_swdge_queues=nq, detect_race_conditions=False)
    x = nc.dram_tensor("x", [H, W, C], f32, kind="ExternalInput")
    xo = nc.dram_tensor("xoff", [H, W, N9], f32, kind="ExternalInput")
    kdin = nc.dram_tensor("kdin", [N9, P, F], f16, kind="ExternalInput")
    mi = nc.dram_tensor("mi_in", [P, H * N9], f32, kind="ExternalInput")
    mr = nc.dram_tensor("mr_in", [P, H * N9], f32, kind="ExternalInput")
    jp = nc.dram_tensor("jp_in", [P, 1], f32, kind="ExternalInput")
    out = nc.dram_tensor("out64", [F, NPIX], f32, kind="ExternalOutput")
    dbg = None
    if debug:
        dbg = {
            "idxw": nc.dram_tensor("idxw_d", [P, SPC * NCALLS], i16,
                                   kind="ExternalOutput").ap(),
            "idxh2": nc.dram_tensor("idxh2_d", [P, MFREE], i16,
                                    kind="ExternalOutput").ap(),
            "idxh": nc.dram_tensor("idxh_d", [P, MFREE], i16,
                                   kind="ExternalOutput").ap(),
            "w4h": nc.dram_tensor("w4h_d", [P, 4 * MFREE], f16,
                                  kind="ExternalOutput").ap(),
        }
        if os.environ.get("DEFCONV_DEBUG") == "2":
            dbg["gdump"] = nc.dram_tensor(
                "gdump_d", [NCALLS * N9 * P, 1024], f16,
                kind="ExternalOutput").ap()
            dbg["sdump"] = nc.dram_tensor(
                "sdump_d", [NCALLS * N9 * P, 1024], f16,
                kind="ExternalOutput").ap()
    with tile.TileContext(nc) as tc:
        _build_kernel(tc, out.ap(),
                      (x.ap(), xo.ap(), kdin.ap(), mi.ap(), mr.ap(), jp.ap()),
                      dbg=dbg)
    nc.compile()
    _CACHE["nc"] = nc
    return nc


def kernel(x_in, y_offset, x_offset, kernel):
    x_in = np.ascontiguousarray(x_in, dtype=np.float32)
    x_offset = np.ascontiguousarray(x_offset, dtype=np.float32)
    kdup, mi, mr, jp1 = _make_consts(np.asarray(kernel, dtype=np.float32))
    nc = _get_nc()
    in_maps = []
    for b in range(NCORES):
        in_maps.append({
            "x": x_in[b],
            "xoff": x_offset[b],
            "kdin": kdup,
            "mi_in": mi,
            "mr_in": mr,
            "jp_in": jp1,
        })
    trace = bool(os.environ.get("DEFCONV_TRACE"))
    kw = {}
    if trace:
        kw["trace"] = True
        td = os.environ.get("DEFCONV_TRACE_DIR")
        if td:
            os.makedirs(td, exist_ok=True)
            kw["tmpdir"] = td
    res = run_bass_kernel_spmd(nc, in_maps, core_ids=list(range(NCORES)), **kw)
    global LAST_EXEC_NS, LAST_TRACE, LAST_RES
    LAST_RES = res
    LAST_EXEC_NS = getattr(res, "exec_time_ns", None) or -1
    it = getattr(res, "instructions_and_trace", None)
    LAST_TRACE = it[1] if it else None
    out = np.empty((NCORES, H, W, F), np.float32)
    for b in range(NCORES):
        o64 = res.results[b]["out64"]            # [F, NPIX] in stream order
        # stream position (cc=ig*4+cg, px=16*jl+iq) -> pixel
        # (i=16*ig+iq, j=32*cg+jl)
        o = o64.reshape(F, 8, 4, 32, 16)         # [f, ig, cg, jl, iq]
        o = o.transpose(1, 4, 2, 3, 0)           # [ig, iq, cg, jl, f]
        out[b] = o.reshape(H, W, F)
    return out



# revision 37
# speedup vs baseline: 1.2438x; 1.2438x over previous
# Self-contained Trainium2 Bass kernel for the deformable-conv problem.
# kernel(**inputs) takes FULL unsharded inputs, shards batch across 8 cores,
# runs one Bass program SPMD, and reassembles the full output.
#
# v2: quad-table entries reordered to (dx, dy, c) so the table builds from an
# interleaved row-pair image with 512B-contiguous runs; WM weight replication
# via stride-0-source DRAM->SBUF HWDGE DMAs (2/call).
# v3: call tiles changed from 4-rows-x-128-cols to 16-rows-x-32-cols so the
# SWDGE index buffer (16-partition-wrapped, 8x core-replicated) is built from
# XBAR-transposed index planes with >=64B-run DMAs instead of the old
# 2B-granular replication chain (8 x 307us serial on the sync queue, the
# dominant cost in v2).  Weight scatter likewise rebuilt with merged (q,n)
# dims at 32B runs.  The per-tap weight multiply moves gpsimd->DVE so gpsimd
# only issues gathers (no more LOAD_LIB/UNLOAD_LIB thrash between the SWDGE
# lib and the TT lib).
import os

import numpy as np

import concourse.bacc as bacc
import concourse.bass as bass
import concourse.mybir as mybir
import concourse.tile as tile
from concourse.tile import add_dep_helper
from concourse.bass_utils import run_bass_kernel_spmd

f32 = mybir.dt.float32
f16 = mybir.dt.float16
i16 = mybir.dt.int16
i32 = mybir.dt.int32
OP = mybir.AluOpType
P = 128
N9 = 9
C = 64
F = 64
H = 128
W = 128
NPIX = H * W
PXB = 512
DI = PXB // P
NCALLS = NPIX // PXB
CALL = PXB * N9
SPC = CALL // 16
HCH = 64
NCH = H // HCH
MCH = HCH * N9
MFREE = H * N9
XCOL_RANK = H + 1
CORNER_RANK = H + 2
NRANKS = H + 3
NSLOTS = NRANKS * 128
GIDX = PXB
GSP = GIDX // 16
NCORES = 8


def _build_kernel(tc, outs, ins, dbg=None):
    nc = tc.nc
    x_in, xoff, kdup, mi_in, mr_in, jp1_in = ins
    out64 = outs

    with tc.tile_pool(name="dram", bufs=1, space="DRAM") as dpool:
        qt_d = dpool.tile([NSLOTS * 256], f16)
        w4_d = dpool.tile([NCALLS * 4 * CALL], f16)
        idx16_d = dpool.tile([16 * SPC * NCALLS], i16)
        qtt, qto = qt_d[:].tensor, qt_d[:].offset

        def S(name, shape, dtype):
            return nc.alloc_sbuf_tensor("sb_" + name, shape, dtype).ap()

        idxw = S("idxw", [P, SPC * NCALLS], i16)
        idxh2 = S("idxh2", [P, MFREE], i16)
        G_ring = [S(f"Gr{k}", [P, 2, PXB], f16) for k in range(8)]
        S_ring = [S(f"Sr{k}", [P, 2, PXB], f16) for k in range(8)]
        WM_ring = [S(f"WMr{k}", [P, 2, CALL], f16) for k in range(2)]
        ps_ring = [nc.alloc_psum_tensor(f"psr{k}", [F, PXB], f32).ap()
                   for k in range(4)]
        ob_ring = [S(f"obr{k}", [F, PXB], f32) for k in range(3)]
        kd = S("kd", [P, N9 * F], f16)
        jp1 = S("jp1", [P, 1], f32)
        w4h = S("w4h", [P, 4 * MFREE], f16)
        idxh = S("idxh", [P, MFREE], i16)
        kdsrc = bass.AP(kdup[:].tensor, kdup[:].offset,
                        [[F, P], [P * F, N9], [1, F]])
        nc.sync.dma_start(out=kd[:], in_=kdsrc)
        nc.sync.dma_start(out=jp1[:], in_=jp1_in[:])

        # Hoist chunk-0 Stage-B input loads ahead of Stage A so the DVE
        # weight/index pipeline starts at ~10us instead of queueing its
        # loads behind the (long) table-build DMAs.
        _tc = {}

        def T(name, dtype=f32):
            if name not in _tc:
                _tc[name] = S(name, [P, MCH], dtype)
            return _tc[name]

        def load_chunk(ich):
            ibase = ich * HCH
            xo = T("xo")
            mi = T("mi")
            mr = T("mr")
            xosrc = bass.AP(xoff[:].tensor, xoff[:].offset + ibase * W * N9,
                            [[N9, W], [W * N9, HCH], [1, N9]])
            nc.sync.dma_start(out=xo[:], in_=xosrc)
            nc.sync.dma_start(out=mi[:], in_=mi_in[:, ibase * N9:(ibase + HCH) * N9])
            nc.sync.dma_start(out=mr[:], in_=mr_in[:, ibase * N9:(ibase + HCH) * N9])

        load_chunk(0)

        # ---- Stage A: fp16 image in SBUF + quad table in DRAM ----
        # Entry element order is (dy, dx, c): elems [0:128] = dy=0 pair
        # [X(y0-1,x0-1), X(y0-1,x0)] — a 256B CONTIGUOUS run of one xh row —
        # elems [128:256] = dy=1 pair from the next row.  After the
        # transpose-gather, partitions = (dx, c) and the free slot = dy; kd
        # is duplicated symmetrically so the matmul contraction is
        # unchanged.  This order builds the table STRAIGHT from xh (no
        # interleaved xq intermediate, whose build was a 170us serial DMA).
        xh = S("xh", [H, W * C], f16)
        XCH = W * C // 4
        xf = S("xf", [H, XCH], f32)
        for ch in range(4):
            xsrc = bass.AP(x_in[:].tensor, x_in[:].offset + ch * XCH,
                           [[W * C, H], [1, XCH]])
            nc.sync.dma_start(out=xf[:], in_=xsrc)
            nc.scalar.copy(out=xh[:, ch * XCH:(ch + 1) * XCH], in_=xf[:])
        zt = S("zt", [P, 512], f16)
        nc.vector.memset(zt[:], 0.0)
        ztp = zt[:].ap[0][0]
        zsrc = lambda rows, elems: bass.AP(zt[:].tensor, zt[:].offset,
                                           [[ztp, rows], [1, elems]])
        xpitch = xh[:].ap[0][0]
        xbase = xh[:].offset
        # Main window (y0 0..127, x0 1..127), split by dy half and by y0
        # half across both HWDGE queues (4 data DMAs, 256B runs from xh).
        for yh in range(2):
            y0a = yh * 64
            # dy=0 half <- xh row y0-1 (y0=0 row is zero-filled below)
            na = 63 if yh == 0 else 64
            ya = 1 if yh == 0 else 64
            nc.sync.dma_start(
                out=bass.AP(qtt, qto + ya * 128 * 256 + 256,
                            [[128 * 256, na], [256, W - 1], [1, 128]]),
                in_=bass.AP(xh[:].tensor, xbase + (ya - 1) * xpitch,
                            [[xpitch, na], [C, W - 1], [1, 128]]))
            # dy=1 half <- xh row y0
            nc.scalar.dma_start(
                out=bass.AP(qtt, qto + y0a * 128 * 256 + 256 + 128,
                            [[128 * 256, 64], [256, W - 1], [1, 128]]),
                in_=bass.AP(xh[:].tensor, xbase + y0a * xpitch,
                            [[xpitch, 64], [C, W - 1], [1, 128]]))
        # y0=0, dy=0 halves are X(-1,.) = 0 (x0 1..127)
        nc.sync.dma_start(out=bass.AP(qtt, qto + 256, [[256, W - 1], [1, 128]]),
                          in_=zsrc(W - 1, 128))
        # x0=0 column (y0 0..127): dx=0 (padded col 0) zero quarters at
        # [0:64] and [128:192]; dx=1 <- xh col 0: dy0 [64:128] from row
        # y0-1 (zero for y0=0), dy1 [192:256] from row y0.
        nc.scalar.dma_start(out=bass.AP(qtt, qto, [[128 * 256, 128], [1, 64]]),
                            in_=zsrc(128, 64))
        nc.sync.dma_start(out=bass.AP(qtt, qto + 128, [[128 * 256, 128], [1, 64]]),
                          in_=zsrc(128, 64))
        nc.scalar.dma_start(
            out=bass.AP(qtt, qto + 128 * 256 + 64, [[128 * 256, 127], [1, 64]]),
            in_=bass.AP(xh[:].tensor, xbase, [[xpitch, 127], [1, 64]]))
        nc.sync.dma_start(out=bass.AP(qtt, qto + 64, [[1, 1], [1, 64]]),
                          in_=zsrc(1, 64))
        nc.scalar.dma_start(
            out=bass.AP(qtt, qto + 192, [[128 * 256, 128], [1, 64]]),
            in_=bass.AP(xh[:].tensor, xbase, [[xpitch, 128], [1, 64]]))
        # rank y0=128 (slots 128*128+x0): dy=0 <- xh row 127, dy=1 = zero.
        nc.sync.dma_start(
            out=bass.AP(qtt, qto + (128 * 128 + 1) * 256, [[256, W - 1], [1, 128]]),
            in_=bass.AP(xh[:].tensor, xbase + 127 * xpitch,
                        [[xpitch, 1], [C, W - 1], [1, 128]]))
        # x0=0 of rank 128: [0:64]=0, [64:128]=xh[127,0:64]
        nc.scalar.dma_start(
            out=bass.AP(qtt, qto + 128 * 128 * 256, [[1, 1], [1, 64]]),
            in_=zsrc(1, 64))
        nc.sync.dma_start(
            out=bass.AP(qtt, qto + 128 * 128 * 256 + 64, [[1, 1], [1, 64]]),
            in_=bass.AP(xh[:].tensor, xbase + 127 * xpitch,
                        [[xpitch, 1], [1, 64]]))
        nc.scalar.dma_start(
            out=bass.AP(qtt, qto + 128 * 128 * 256 + 128, [[256, 128], [1, 128]]),
            in_=zsrc(128, 128))
        # xcol rank (x0=128, slots 129*128+y0): dx=0 <- xh col 127; dx=1
        # (padded col 129) zero quarters.
        nc.sync.dma_start(
            out=bass.AP(qtt, qto + (XCOL_RANK * 128 + 1) * 256,
                        [[256, 127], [1, 64]]),
            in_=bass.AP(xh[:].tensor, xbase + (W - 1) * C,
                        [[xpitch, 127], [1, 64]]))
        nc.scalar.dma_start(
            out=bass.AP(qtt, qto + XCOL_RANK * 128 * 256, [[1, 1], [1, 64]]),
            in_=zsrc(1, 64))
        nc.sync.dma_start(
            out=bass.AP(qtt, qto + XCOL_RANK * 128 * 256 + 128,
                        [[256, 128], [1, 64]]),
            in_=bass.AP(xh[:].tensor, xbase + (W - 1) * C,
                        [[xpitch, 128], [1, 64]]))
        nc.scalar.dma_start(
            out=bass.AP(qtt, qto + XCOL_RANK * 128 * 256 + 64,
                        [[256, 128], [1, 64]]),
            in_=zsrc(128, 64))
        nc.sync.dma_start(
            out=bass.AP(qtt, qto + XCOL_RANK * 128 * 256 + 192,
                        [[256, 128], [1, 64]]),
            in_=zsrc(128, 64))
        # corner (x0=128, y0=128): [0:64] = X(127,127), [64:256] = 0
        nc.scalar.dma_start(
            out=bass.AP(qtt, qto + CORNER_RANK * 128 * 256, [[1, 1], [1, C]]),
            in_=bass.AP(xh[:].tensor, xbase + 127 * xpitch + (W - 1) * C,
                        [[xpitch, 1], [1, C]]))
        nc.sync.dma_start(
            out=bass.AP(qtt, qto + CORNER_RANK * 128 * 256 + 64, [[1, 1], [1, 192]]),
            in_=zsrc(1, 192))

        # ---- Stage B: indices & weights ----
        w4h4 = w4h[:].rearrange("p (q n i) -> p q n i", q=4, n=N9, i=H)
        idxh3 = idxh[:].rearrange("p (n i) -> p n i", n=N9, i=H)

        for ich in range(NCH):
            ibase = ich * HCH
            if ich > 0:
                load_chunk(ich)
            xo = T("xo")
            mi = T("mi")
            mr = T("mr")

            def side(pre, base_is_j):
                # rel/ti/tf/hi/lo are dead once side() returns — share one
                # scratch set between the x and y passes (SBUF pressure).
                rel = T("srel")
                if base_is_j:
                    nc.vector.tensor_scalar(out=rel[:], in0=xo[:], scalar1=jp1[:, 0:1],
                                            scalar2=None, op0=OP.add)
                else:
                    nc.vector.tensor_tensor(out=rel[:], in0=xo[:], in1=mi[:], op=OP.add)
                nc.vector.tensor_tensor(out=rel[:], in0=rel[:], in1=mr[:], op=OP.add)
                ti = T("sti", i32)
                tf = T("stf")
                nc.vector.tensor_copy(out=ti[:], in_=rel[:])
                nc.vector.tensor_copy(out=tf[:], in_=ti[:])
                corr = T(pre + "corr")
                nc.vector.tensor_tensor(out=corr[:], in0=tf[:], in1=rel[:], op=OP.is_gt)
                nc.vector.tensor_tensor(out=tf[:], in0=tf[:], in1=corr[:], op=OP.subtract)
                r0 = tf
                dim = W if base_is_j else H
                c0 = T(pre + "c0")
                nc.vector.tensor_scalar(out=c0[:], in0=r0[:], scalar1=0.0,
                                        scalar2=float(dim + 1), op0=OP.max, op1=OP.min)
                c1 = T(pre + "c1")
                nc.vector.tensor_scalar(out=c1[:], in0=r0[:], scalar1=1.0, scalar2=0.0,
                                        op0=OP.add, op1=OP.max)
                nc.vector.tensor_scalar(out=c1[:], in0=c1[:], scalar1=float(dim + 1),
                                        scalar2=None, op0=OP.min)
                nc.vector.tensor_tensor(out=c1[:], in0=c1[:], in1=rel[:], op=OP.subtract)
                w0 = c1
                nc.vector.tensor_tensor(out=c0[:], in0=rel[:], in1=c0[:], op=OP.subtract)
                w1 = c0
                hi = T("shi")
                nc.vector.tensor_scalar(out=hi[:], in0=r0[:], scalar1=float(dim + 1),
                                        scalar2=None, op0=OP.is_ge)
                lo = T("slo")
                nc.vector.tensor_scalar(out=lo[:], in0=r0[:], scalar1=-1.0,
                                        scalar2=None, op0=OP.is_le)
                nc.vector.tensor_tensor(out=hi[:], in0=w0[:], in1=hi[:], op=OP.mult)
                t0 = hi
                nc.vector.tensor_tensor(out=lo[:], in0=w1[:], in1=lo[:], op=OP.mult)
                t1 = lo
                nc.vector.tensor_tensor(out=w0[:], in0=w0[:], in1=t0[:], op=OP.subtract)
                nc.vector.tensor_tensor(out=w0[:], in0=w0[:], in1=t1[:], op=OP.add)
                nc.vector.tensor_tensor(out=w1[:], in0=w1[:], in1=t1[:], op=OP.subtract)
                nc.vector.tensor_tensor(out=w1[:], in0=w1[:], in1=t0[:], op=OP.add)
                nc.vector.tensor_scalar(out=corr[:], in0=r0[:], scalar1=0.0,
                                        scalar2=float(dim), op0=OP.max, op1=OP.min)
                return w0, w1, corr

            B0, B1, bx = side("x", True)
            A0, A1, by = side("y", False)

            prod = T("prod")
            for qi, (ay, bw) in enumerate(((A0, B0), (A0, B1), (A1, B0), (A1, B1))):
                nc.vector.tensor_tensor(out=prod[:], in0=ay[:], in1=bw[:], op=OP.mult)
                nc.vector.tensor_copy(
                    out=w4h4[:, qi, :, ibase:ibase + HCH],
                    in_=prod[:].rearrange("p (i n) -> p n i", i=HCH, n=N9))

            m128 = T("srel")
            nc.vector.tensor_scalar(out=m128[:], in0=bx[:], scalar1=float(W),
                                    scalar2=None, op0=OP.is_ge)
            idxa = T("stf")
            nc.vector.tensor_scalar(out=idxa[:], in0=by[:], scalar1=128.0,
                                    scalar2=None, op0=OP.mult)
            nc.vector.tensor_tensor(out=idxa[:], in0=idxa[:], in1=bx[:], op=OP.add)
            idxb = T("shi")
            nc.vector.tensor_scalar(out=idxb[:], in0=by[:],
                                    scalar1=float(XCOL_RANK * 128),
                                    scalar2=None, op0=OP.add)
            nc.vector.tensor_tensor(out=idxb[:], in0=idxb[:], in1=idxa[:], op=OP.subtract)
            nc.vector.tensor_tensor(out=idxb[:], in0=idxb[:], in1=m128[:], op=OP.mult)
            nc.vector.tensor_tensor(out=idxa[:], in0=idxa[:], in1=idxb[:], op=OP.add)
            nc.vector.tensor_copy(
                out=idxh3[:, :, ibase:ibase + HCH],
                in_=idxa[:].rearrange("p (i n) -> p n i", i=HCH, n=N9))

        # Call cc = ig*4+cg covers rows [16ig,16ig+16) x cols [32cg,32cg+32);
        # SWDGE enumerates px = 16*jl + iq (iq = idx-partition-within-16,
        # jl = free slot), so pixel (16ig+iq, 32cg+jl) sits at px.
        #
        # idxw path: XBAR-transpose the 9 idx planes [p=j, i] -> idxh2
        # [p=i, j], scatter to DRAM in the 16-wrapped order (64B runs),
        # then 8 fully-contiguous replication DMAs (one per gpsimd core
        # group, 16 descriptors x 18KB each).
        w4p = w4h[:].ap[0][0]
        idxp = idxh[:].ap[0][0]
        i2p = idxh2[:].ap[0][0]
        for n in range(N9):
            eng = nc.sync if n % 2 else nc.scalar
            eng.dma_start_transpose(
                out=idxh2[:, n * H:(n + 1) * H],
                in_=idxh[:, n * H:(n + 1) * H])
        for ig in range(8):
            for cg in range(4):
                eng = nc.sync if (ig * 4 + cg) % 2 else nc.scalar
                src = bass.AP(idxh2[:].tensor,
                              idxh2[:].offset + 16 * ig * i2p + 32 * cg,
                              [[i2p, 16], [H, N9], [1, 32]])
                dst = bass.AP(idx16_d[:].tensor,
                              idx16_d[:].offset + (ig * 4 + cg) * SPC,
                              [[SPC * NCALLS, 16], [32, N9], [1, 32]])
                eng.dma_start(out=dst, in_=src)
        ipitch = idxw[:].ap[0][0]
        for rep in range(8):
            iwdst = bass.AP(idxw[:].tensor, idxw[:].offset + rep * 16 * ipitch,
                            [[ipitch, 16], [1, SPC * NCALLS]])
            iwsrc = bass.AP(idx16_d[:].tensor, idx16_d[:].offset,
                            [[SPC * NCALLS, 16], [1, SPC * NCALLS]])
            nc.sync.dma_start(out=iwdst, in_=iwsrc)
        # w4 scatter: dst elem = cc*4*CALL + (2*dy+dx)*CALL + n*PXB + 16*jl
        # + iq; (q4,n) merge to one stride-512 level on both sides.
        for ig in range(8):
            for cg in range(4):
                eng = nc.sync if (ig * 4 + cg) % 2 else nc.scalar
                src = bass.AP(w4h[:].tensor,
                              w4h[:].offset + 32 * cg * w4p + 16 * ig,
                              [[w4p, 32], [H, 36], [1, 16]])
                dst = bass.AP(w4_d[:].tensor,
                              w4_d[:].offset + (ig * 4 + cg) * 4 * CALL,
                              [[16, 32], [PXB, 36], [1, 16]])
                eng.dma_start(out=dst, in_=src)
        if dbg is not None:
            nc.sync.dma_start(out=dbg["idxw"][:], in_=idxw[:])
            nc.scalar.dma_start(out=dbg["idxh2"][:], in_=idxh2[:])
            nc.sync.dma_start(out=dbg["idxh"][:], in_=idxh[:])
            nc.scalar.dma_start(out=dbg["w4h"][:], in_=w4h[:])

        # ---- main loop ----
        # WM[p=(dy,c), dx, n*PXB+px] = w(dy,dx,n,px), built by a single
        # stride-0-source DRAM->SBUF DMA per dy half (64 replicated reads
        # of the same 9216B row -> 128 contiguous descriptors). No
        # SBUF->SBUF DMA in the loop, so gathers need no serialization.
        qtv = qt_d[:].rearrange("(s e) -> s e", s=NSLOTS, e=256)
        ob_insts = [None, None, None, None]
        use_dve = os.environ.get("DEFCONV_TT_ENG", "vector") != "gpsimd"

        def emit_tt_mm(cc, n, k, xbar_margin_gi):
            # TT for tap k; on DVE it additionally waits for gather k+1's
            # DMA so the XBAR transpose-write of gather k has fully landed
            # before DVE touches it (gpsimd consumers are slow enough to
            # never expose that window; DVE is not).
            WM = WM_ring[cc % 2]
            wmp = WM[:].ap[0][0]
            Sg = S_ring[k % 8]
            G = G_ring[k % 8]
            wmin = bass.AP(WM[:].tensor, WM[:].offset + n * PXB,
                           [[wmp, P], [CALL, 2], [1, PXB]])
            eng = nc.vector if (use_dve and xbar_margin_gi is not None) \
                else nc.gpsimd
            tti = eng.tensor_tensor(out=Sg[:], in0=G[:], in1=wmin, op=OP.mult)
            if use_dve and xbar_margin_gi is not None:
                add_dep_helper(tti.ins, xbar_margin_gi.ins, True, "xbar margin")
            if dbg is not None and "sdump" in dbg:
                st, so = dbg["sdump"][:].tensor, dbg["sdump"][:].offset
                nc.scalar.dma_start(
                    out=bass.AP(st, so + k * P * 1024, [[1024, P], [1, 1024]]),
                    in_=Sg[:])
            ps = ps_ring[cc % 4]
            lhsT = kd[:, n * F:(n + 1) * F]
            for dx in range(2):
                mmi = nc.tensor.matmul(
                    ps[:], lhsT, Sg[:, dx, :],
                    start=(n == 0 and dx == 0),
                    stop=(n == N9 - 1 and dx == 1))
                if n == 0 and dx == 0 and ob_insts[cc % 4] is not None:
                    add_dep_helper(mmi.ins, ob_insts[cc % 4].ins, True, "psum reuse")
            if n == N9 - 1:
                ob = ob_ring[cc % 3]
                obi = nc.scalar.copy(out=ob[:], in_=ps[:])
                ob_insts[cc % 4] = obi
                nc.sync.dma_start(out=out64[:, cc * PXB:(cc + 1) * PXB], in_=ob[:])

        pend = None
        for cc in range(NCALLS):
            WM = WM_ring[cc % 2]
            wmp = WM[:].ap[0][0]
            # partitions are (dx, c) now and the gather free half = dy:
            # partition half dxh holds q4 = 2*dy + dxh at free half dy,
            # so the src walks dy with stride 2*CALL.
            for dxh in range(2):
                wmdst = bass.AP(WM[:].tensor, WM[:].offset + dxh * C * wmp,
                                [[wmp, C], [CALL, 2], [1, CALL]])
                wmsrc = bass.AP(w4_d[:].tensor,
                                w4_d[:].offset + (cc * 4 + dxh) * CALL,
                                [[0, C], [2 * CALL, 2], [1, CALL]])
                nc.scalar.dma_start(out=wmdst, in_=wmsrc)
            for n in range(N9):
                k = cc * N9 + n
                gi = nc.gpsimd.dma_gather(
                    G_ring[k % 8][:], qtv,
                    idxw[:, cc * SPC + n * GSP: cc * SPC + (n + 1) * GSP],
                    num_idxs=GIDX, num_idxs_reg=GIDX, elem_size=256,
                    transpose=True,
                    queue_num=k % int(os.environ.get("DEFCONV_NQ", "1")))
                if dbg is not None and "gdump" in dbg:
                    gt, go = dbg["gdump"][:].tensor, dbg["gdump"][:].offset
                    nc.sync.dma_start(
                        out=bass.AP(gt, go + k * P * 1024,
                                    [[1024, P], [1, 1024]]),
                        in_=G_ring[k % 8][:])
                if pend is not None:
                    emit_tt_mm(*pend, xbar_margin_gi=gi)
                pend = (cc, n, k)
        emit_tt_mm(*pend, xbar_margin_gi=None)


def _make_consts(kernel_np):
    k9 = kernel_np.reshape(N9, C, F)
    kdup = np.concatenate([k9, k9], axis=1).astype(np.float16)
    ii = np.repeat(np.arange(H, dtype=np.float32) + 1.0, N9)
    R = np.tile(np.arange(-1, 2, dtype=np.float32), 3)
    rr = np.tile(R, H)
    mi = np.broadcast_to(ii, (P, H * N9)).copy()
    mr = np.broadcast_to(rr, (P, H * N9)).copy()
    jp1 = (np.arange(P, dtype=np.float32) + 1.0).reshape(P, 1)
    return kdup, mi, mr, jp1


_CACHE = {}


def _get_nc():
    if "nc" in _CACHE:
        return _CACHE["nc"]
    debug = bool(os.environ.get("DEFCONV_DEBUG"))
    # num_swdge_queues MUST stay 1: concurrent SWDGE transpose-gather DMAs
    # on >=2 queues corrupt the gathered data (shared XBAR transpose-RX
    # hazard, reproduced at NQ=2 and NQ=4 with px%16-lane-granular garbage).
    nq = int(os.environ.get("DEFCONV_NQ", "1"))
    nc = bacc.Bacc("TRN2", target_bir_lowering=False, debug=False,
                   num_swdge_queues=nq, detect_race_conditions=False)
    x = nc.dram_tensor("x", [H, W, C], f32, kind="ExternalInput")
    xo = nc.dram_tensor("xoff", [H, W, N9], f32, kind="ExternalInput")
    kdin = nc.dram_tensor("kdin", [N9, P, F], f16, kind="ExternalInput")
    mi = nc.dram_tensor("mi_in", [P, H * N9], f32, kind="ExternalInput")
    mr = nc.dram_tensor("mr_in", [P, H * N9], f32, kind="ExternalInput")
    jp = nc.dram_tensor("jp_in", [P, 1], f32, kind="ExternalInput")
    out = nc.dram_tensor("out64", [F, NPIX], f32, kind="ExternalOutput")
    dbg = None
    if debug:
        dbg = {
            "idxw": nc.dram_tensor("idxw_d", [P, SPC * NCALLS], i16,
                                   kind="ExternalOutput").ap(),
            "idxh2": nc.dram_tensor("idxh2_d", [P, MFREE], i16,
                                    kind="ExternalOutput").ap(),
            "idxh": nc.dram_tensor("idxh_d", [P, MFREE], i16,
                                   kind="ExternalOutput").ap(),
            "w4h": nc.dram_tensor("w4h_d", [P, 4 * MFREE], f16,
                                  kind="ExternalOutput").ap(),
        }
        if os.environ.get("DEFCONV_DEBUG") == "2":
            dbg["gdump"] = nc.dram_tensor(
                "gdump_d", [NCALLS * N9 * P, 1024], f16,
                kind="ExternalOutput").ap()
            dbg["sdump"] = nc.dram_tensor(
                "sdump_d", [NCALLS * N9 * P, 1024], f16,
                kind="ExternalOutput").ap()
    with tile.TileContext(nc) as tc:
        _build_kernel(tc, out.ap(),
                      (x.ap(), xo.ap(), kdin.ap(), mi.ap(), mr.ap(), jp.ap()),
                      dbg=dbg)
    nc.compile()
    _CACHE["nc"] = nc
    return nc


def kernel(x_in, y_offset, x_offset, kernel):
    x_in = np.ascontiguousarray(x_in, dtype=np.float32)
    x_offset = np.ascontiguousarray(x_offset, dtype=np.float32)
    kdup, mi, mr, jp1 = _make_consts(np.asarray(kernel, dtype=np.float32))
    nc = _get_nc()
    in_maps = []
    for b in range(NCORES):
        in_maps.append({
            "x": x_in[b],
            "xoff": x_offset[b],
            "kdin": kdup,
            "mi_in": mi,
            "mr_in": mr,
            "jp_in": jp1,
        })
    trace = bool(os.environ.get("DEFCONV_TRACE"))
    kw = {}
    if trace:
        kw["trace"] = True
        td = os.environ.get("DEFCONV_TRACE_DIR")
        if td:
            os.makedirs(td, exist_ok=True)
            kw["tmpdir"] = td
    res = run_bass_kernel_spmd(nc, in_maps, core_ids=list(range(NCORES)), **kw)
    global LAST_EXEC_NS, LAST_TRACE, LAST_RES
    LAST_RES = res
    LAST_EXEC_NS = getattr(res, "exec_time_ns", None) or -1
    it = getattr(res, "instructions_and_trace", None)
    LAST_TRACE = it[1] if it else None
    out = np.empty((NCORES, H, W, F), np.float32)
    for b in range(NCORES):
        o64 = res.results[b]["out64"]            # [F, NPIX] in stream order
        # stream position (cc=ig*4+cg, px=16*jl+iq) -> pixel
        # (i=16*ig+iq, j=32*cg+jl)
        o = o64.reshape(F, 8, 4, 32, 16)         # [f, ig, cg, jl, iq]
        o = o.transpose(1, 4, 2, 3, 0)           # [ig, iq, cg, jl, f]
        out[b] = o.reshape(H, W, F)
    return out

